# revision 48
# baseline (speedup 1.0000x reference)
"""MHSA + RoPE kernel for Trainium2, 8 NeuronCores.

Sharding: data-parallel over batch (B=2) x tensor-parallel over heads
(16 heads -> 4 head-groups of 4). Core c handles batch c//4, heads
[4*(c%4) : 4*(c%4)+4]. Each core computes its partial o_proj output
[N, D]; host sums the 4 partials per batch (the "all-reduce").

Per-core schedule (single TileContext scope, per-head pipeline so the
Tile scheduler can fill attention's ACT-bound PE gaps with the next
head's projection matmuls and keep the PE HAM clock gate warm):

  h=0: k0,q0 proj (+inline RoPE chunks) -> v proj (all heads) -> attn0
  h>0: k_h,q_h proj + RoPE (overlaps attn_{h-1}) -> attn_h
  o_proj at the end (overlaps attn3 via the scheduler).

RoPE is applied to [128,1024] column chunks right after the projection
eviction that produces them, so the rope->scores dependency chain at a
head boundary is ~2us instead of ~10us.

Softmax denominators: an all-ones [128,128] matmul partition-reduces
acc AND broadcasts the result to all partitions in one shot; the
reciprocal runs as reciprocal_approx_fast (single DVE uop chain, ~5x
faster than reciprocal()); the normalize multiply reads a_ps (PSUM) x
bc (SBUF) on DVE.

Everything on-chip is fp16 (same PE rate as bf16, 2x DVE mode, half
the SBUF/DMA of f32, 11-bit mantissa: exp values <= ~200 and softmax
denominators ~3e3 are represented to ~0.05%). PSUM stays f32.

PSUM budget (8 banks): scores [128,1024]x2 bufs = 4, PV accumulator
[128,1024]x1 = 2, shared proj/tail/o_proj pool [128,512]x2 = 2.
o_proj additionally reuses the scores pool slots once attention ends.
"""

import sys

sys.path.insert(0, "/opt/trn_rl_repo")

import numpy as np

import concourse.bass as bass
import concourse.tile as tile
from concourse import bacc, mybir
from concourse.bass_utils import run_bass_kernel_spmd

F32 = mybir.dt.float32
F16 = mybir.dt.float16
MULT = mybir.AluOpType.mult
ADD = mybir.AluOpType.add
EXP = mybir.ActivationFunctionType.Exp
PSUM = bass.MemorySpace.PSUM

B, N, D = 2, 2048, 2048
H, HD = 16, 128
HL = 4            # local heads per core
C = HL * HD       # 512 local head cols
KT = D // 128     # 16 contraction tiles
NB = 4            # n-blocks of 512 for projections
NT = N // 128     # 16 j-tiles
SCALE = float(HD) ** -0.5
N_CORES = 8

_CACHE = {}


def _build_program():
    nc = bacc.Bacc("TRN2", target_bir_lowering=False, debug=False,
                   num_devices=N_CORES)

    xt_d = nc.dram_tensor("xt", [NB, 128, KT, 512], F16, kind="ExternalInput")
    # wq/wk are head-major so head 0's slice (512KB) can load alone in
    # the DMA-bound startup window; heads 1-3 defer until attention 0
    wq_d = nc.dram_tensor("wq", [128, HL, KT, 128], F16,
                          kind="ExternalInput")
    wk_d = nc.dram_tensor("wk", [128, HL, KT, 128], F16,
                          kind="ExternalInput")
    wv_d = nc.dram_tensor("wv", [128, KT, C], F16, kind="ExternalInput")
    wo_d = nc.dram_tensor("wo", [128, HL, D], F16, kind="ExternalInput")
    cos_d = nc.dram_tensor("cos", [128, N], F16, kind="ExternalInput")
    sin_d = nc.dram_tensor("sin", [128, N], F16, kind="ExternalInput")
    onem_d = nc.dram_tensor("onem", [128, 128], F16, kind="ExternalInput")
    out_d = nc.dram_tensor("out", [N, D], F16, kind="ExternalOutput")

    with tile.TileContext(nc) as tc:
        with (
            tc.tile_pool(name="res", bufs=1) as res,
            tc.tile_pool(name="qk", bufs=2) as qkp,
            tc.tile_pool(name="rope", bufs=2) as ropep,
            tc.tile_pool(name="sx", bufs=4) as sxp,
            tc.tile_pool(name="accp", bufs=2) as accp,
            tc.tile_pool(name="pp", bufs=2, space=PSUM) as pp,
            tc.tile_pool(name="sps", bufs=2, space=PSUM) as sps,
            tc.tile_pool(name="aps", bufs=2, space=PSUM) as aps,
        ):
            vv = res.tile([128, NT, C], F16)      # v natural [n, c]
            ao = res.tile([128, HL, N], F16)      # normalized A^T [c, n]
            cos_sb = res.tile([128, N], F16)
            sin_sb = res.tile([128, N], F16)
            onem = res.tile([128, 128], F16)

            def rope_chunk(dst, lo):
                # in-place RoPE on dst[:, lo:lo+1024]; sin sign-folded
                # on host. The d-half swap is a partition shuffle ->
                # SBUF-SBUF DMA.
                sl = slice(lo, lo + 1024)
                tmp = ropep.tile([128, 1024], F16, tag="tmp")
                nc.sync.dma_start(tmp[0:64, :], dst[64:128, sl])
                nc.sync.dma_start(tmp[64:128, :], dst[0:64, sl])
                nc.vector.tensor_tensor(tmp[:], tmp[:], sin_sb[:, sl],
                                        op=MULT)
                nc.vector.tensor_tensor(dst[:, sl], dst[:, sl],
                                        cos_sb[:, sl], op=MULT)
                nc.vector.tensor_tensor(dst[:, sl], dst[:, sl], tmp[:],
                                        op=ADD)

            with tc.tile_pool(name="wp", bufs=1) as wp:
                x_sb = wp.tile([128, NB, KT, 512], F16, tag="x")
                wq_sb = wp.tile([128, HL, KT, 128], F16, tag="wq")
                wk_sb = wp.tile([128, HL, KT, 128], F16, tag="wk")
                wv_sb = wp.tile([128, KT, C], F16, tag="wv")

                # DMA order matches consumption order (k01, q01, v
                # first half, k23, q23, v rest). Half-tensor (1MB)
                # transfers: small per-ktile pieces measured ~200GB/s
                # vs ~430GB/s for large ones, so split no finer than
                # halves, interleaved to spread across DMA queues.
                # Alternate issue between the two HWDGE engines (sync
                # and scalar, both idle at start): each dma_start costs
                # ~600ns of issue time on its engine, so a single-queue
                # burst of 17 serializes ~10us of issue latency.
                startup = [
                    (wk_sb[:, 0], wk_d[:, 0]),
                    (x_sb[:, 0, 0:8], xt_d[0, :, 0:8]),
                    (x_sb[:, 0, 8:16], xt_d[0, :, 8:16]),
                    (wq_sb[:, 0], wq_d[:, 0]),
                    (x_sb[:, 1, 0:8], xt_d[1, :, 0:8]),
                    (x_sb[:, 1, 8:16], xt_d[1, :, 8:16]),
                    (wv_sb[:, 0:8], wv_d[:, 0:8]),
                    (wv_sb[:, 8:16], wv_d[:, 8:16]),
                    (cos_sb[:], cos_d[:]),
                    (sin_sb[:], sin_d[:]),
                    (onem[:], onem_d[:]),
                    (x_sb[:, 2, 0:8], xt_d[2, :, 0:8]),
                    (x_sb[:, 2, 8:16], xt_d[2, :, 8:16]),
                    (x_sb[:, 3, 0:8], xt_d[3, :, 0:8]),
                    (x_sb[:, 3, 8:16], xt_d[3, :, 8:16]),
                    # heads 1-3 of wk/wq aren't touched until the h=1
                    # projections (~65us in): deferred out of the
                    # DMA-bound startup window
                    (wk_sb[:, 1:4], wk_d[:, 1:4]),
                    (wq_sb[:, 1:4], wq_d[:, 1:4]),
                ]
                for dst, src in startup:
                    nc.sync.dma_start(dst, src)

                # Warm the ACT exp table (~2.7us) during the startup
                # DMAs so the first attention exp doesn't eat the load.
                warm = sxp.tile([128, 128], F16, tag="sx")
                nc.scalar.activation(warm[:], cos_sb[:, 0:128], EXP)

                # Warm the PE HAM clock gate during the 0-9us DMA wait:
                # ~5us of discarded matmuls on an (uninitialized)
                # scratch tile opens the 2.4GHz gate before real work
                # arrives, instead of running the first ~3.4us of
                # projections at the cold 1.2GHz rate. Results land in
                # pp slots that are immediately recycled; real
                # accumulations start=True-clear the banks.
                wrm = res.tile([128, 512], F16)
                nc.gpsimd.memset(wrm[:], 0.0)
                for _ in range(12):
                    wps = pp.tile([128, 512], F32, tag="pp")
                    nc.tensor.matmul(wps[:], wrm[:, 0:128], wrm[:],
                                     start=True, stop=True)

                for h in range(HL):
                    # ---- k/q projections for head h: k^T/q^T [d, n],
                    # RoPE chunks inline after the evictions that
                    # complete each 1024-column half. k first so scores
                    # j-tiles unblock as early as possible.
                    qr = qkp.tile([128, N], F16, tag="qr")
                    kr = qkp.tile([128, N], F16, tag="kr")
                    if h == 0:
                        # startup: order matches DMA arrival (wk+x0,
                        # x1, wq, wv, x2, x3) so the PE is never
                        # waiting on a transfer it doesn't need yet,
                        # and attention h0 (which needs kr/qr chunk 0
                        # roped + vv[0..7]) can start early.
                        order = ([("qk", kr, wk_sb, 0),
                                  ("qk", kr, wk_sb, 1),
                                  ("qk", qr, wq_sb, 0),
                                  ("qk", qr, wq_sb, 1)]
                                 + [("v", m) for m in range(8)]
                                 + [("qk", kr, wk_sb, 2),
                                    ("qk", kr, wk_sb, 3),
                                    ("qk", qr, wq_sb, 2),
                                    ("qk", qr, wq_sb, 3)]
                                 + [("v", m) for m in range(8, NT)])
                    else:
                        order = [("qk", kr, wk_sb, nb)
                                 for nb in range(NB)] + \
                                [("qk", qr, wq_sb, nb)
                                 for nb in range(NB)]
                    for item in order:
                        if item[0] == "qk":
                            _, dst, w_sb, nb = item
                            ps = pp.tile([128, 512], F32, tag="pp")
                            for t in range(KT):
                                nc.tensor.matmul(
                                    ps[:],
                                    w_sb[:, h, t, :],
                                    x_sb[:, nb, t, :],
                                    start=(t == 0), stop=(t == KT - 1),
                                )
                            nc.scalar.copy(dst[:, bass.ts(nb, 512)],
                                           ps[:])
                            if nb % 2 == 1:
                                rope_chunk(dst, (nb - 1) * 512)
                        else:
                            # ---- v projection, all heads: v [n, c] ---
                            _, m = item
                            nb, mm = m // 4, m % 4
                            ps = pp.tile([128, 512], F32, tag="pp")
                            for t in range(KT):
                                nc.tensor.matmul(
                                    ps[:],
                                    x_sb[:, nb, t, bass.ts(mm, 128)],
                                    wv_sb[:, t, :],
                                    start=(t == 0), stop=(t == KT - 1),
                                )
                            nc.scalar.copy(vv[:, m, :], ps[:])

                    # ---- attention for head h ------------------------
                    for ih in range(2):
                        ihb = ih * 1024
                        # per-512-col accumulator tiles (1 bank each,
                        # 2 bufs): normalize of f=0 can release its
                        # bank while f=1 still accumulates, halving the
                        # ih-boundary WAR stall.
                        a_ps0 = aps.tile([128, 512], F32, tag="a")
                        a_ps1 = aps.tile([128, 512], F32, tag="a")
                        a_ps = (a_ps0, a_ps1)
                        acc = accp.tile([128, 1024], F16, tag="acc")
                        # For the very last tail (nothing left to fill
                        # PE gaps with), shorten the post-exp critical
                        # chain: the last two j-tiles skip the serial
                        # DVE accumulate and instead contribute to the
                        # denominator via PSUM-accumulated matmuls.
                        last_tail = (h == HL - 1 and ih == 1)
                        tail_exps = []
                        for j in range(NT):
                            s_ps = sps.tile([128, 1024], F32, tag="s")
                            for f in range(2):
                                nc.tensor.matmul(
                                    s_ps[:, bass.ts(f, 512)],
                                    kr[:, bass.ts(j, 128)],
                                    qr[:, ihb + f * 512:
                                        ihb + (f + 1) * 512],
                                    start=True, stop=True,
                                )
                            s_exp = sxp.tile([128, 1024], F16, tag="sx")
                            nc.scalar.activation(s_exp[:], s_ps[:], EXP,
                                                 scale=SCALE)
                            if last_tail and j >= NT - 2:
                                tail_exps.append(s_exp)
                            elif j == 0:
                                nc.vector.tensor_copy(acc[:], s_exp[:])
                            else:
                                nc.vector.tensor_tensor(acc[:], acc[:],
                                                        s_exp[:], op=ADD)
                            for f in range(2):
                                nc.tensor.matmul(
                                    a_ps[f][:],
                                    vv[:, j, bass.ts(h, 128)],
                                    s_exp[:, bass.ts(f, 512)],
                                    start=(j == 0), stop=(j == NT - 1),
                                )
                        # softmax denominators: the all-ones [128,128]
                        # matmul partition-reduces acc AND broadcasts
                        # den[i] to every partition; fast approx
                        # reciprocal; normalize on DVE.
                        for f in range(2):
                            # use a scores-pool slot (fast-cycling, not
                            # the pp slots that next-head proj groups
                            # need to fill the ih-boundary gap)
                            den_ps = sps.tile([128, 512], F32, tag="s")
                            nc.tensor.matmul(den_ps[:], onem[:],
                                             acc[:, bass.ts(f, 512)],
                                             start=True,
                                             stop=not tail_exps)
                            for ti, te in enumerate(tail_exps):
                                nc.tensor.matmul(
                                    den_ps[:], onem[:],
                                    te[:, bass.ts(f, 512)],
                                    start=False,
                                    stop=(ti == len(tail_exps) - 1))
                            bc_sb = accp.tile([128, 512], F32, tag="bc")
                            with nc.allow_low_precision(
                                    reason="softmax denominators: approx "
                                           "recip is ~51 ULP"):
                                nc.vector.reciprocal_approx_fast(
                                    out=bc_sb[:], in_=den_ps[:])
                            nc.vector.tensor_tensor(
                                ao[:, h, ihb + f * 512:
                                   ihb + (f + 1) * 512],
                                a_ps[f][:], bc_sb[:],
                                op=MULT)

            # ---- o_proj (wo/st pools reuse the closed wp space) ------
            with (
                tc.tile_pool(name="op", bufs=1) as op,
                tc.tile_pool(name="stp", bufs=3) as stp,
            ):
                wo_sb = op.tile([128, HL, D], F16, tag="wo")
                nc.sync.dma_start(wo_sb[:], wo_d[:])
                for m in range(NT):
                    st = stp.tile([128, D], F16, tag="st")
                    for f in range(4):
                        # spread o_ps over the pp pool and the (now
                        # idle) scores pool for 4-deep PSUM pipelining
                        if f % 2 == 0:
                            o_ps = pp.tile([128, 512], F32, tag="pp")
                        else:
                            o_ps = sps.tile([128, 512], F32, tag="s")
                        for ct in range(HL):
                            nc.tensor.matmul(
                                o_ps[:],
                                ao[:, ct, bass.ts(m, 128)],
                                wo_sb[:, ct, bass.ts(f, 512)],
                                start=(ct == 0), stop=(ct == HL - 1),
                            )
                        # alternate eviction engines so neither paces
                        # o_proj
                        if f % 2 == 0:
                            nc.scalar.copy(st[:, bass.ts(f, 512)],
                                           o_ps[:])
                        else:
                            nc.vector.tensor_copy(st[:, bass.ts(f, 512)],
                                                  o_ps[:])
                        # store each 512-col slice as soon as its
                        # eviction lands (shrinks the final-store tail)
                        nc.sync.dma_start(
                            out_d[bass.ts(m, 128), bass.ts(f, 512)],
                            st[:, bass.ts(f, 512)])

    nc.compile()
    return nc


def _rope_tables():
    inv_freq = 1.0 / (10000.0 ** (np.arange(0, HD, 2, dtype=np.float32) / HD))
    pos = np.arange(N, dtype=np.float32)
    freqs = pos[:, None] * inv_freq[None, :]          # [N, HD/2]
    emb = np.concatenate([freqs, freqs], axis=-1)     # [N, HD]
    cos = np.cos(emb).astype(np.float32).T.copy()     # [HD, N]
    sin = np.sin(emb).astype(np.float32).T.copy()     # [HD, N]
    sin_signed = sin.copy()
    sin_signed[0:64] *= -1.0
    return cos, sin_signed


def _make_in_maps(x, Wq, Wk, Wv, Wo):
    cos, sin_signed = _rope_tables()
    f16 = np.float16

    in_maps = []
    for c in range(N_CORES):
        b, hg = c // 4, c % 4
        cols = slice(C * hg, C * hg + C)
        xT = np.ascontiguousarray(x[b].T)                      # [D, N]
        xt = np.ascontiguousarray(
            xT.reshape(KT, 128, NB, 512).transpose(2, 1, 0, 3)
        ).astype(f16)                                          # [NB,128,KT,512]

        def wslice(W):
            wt = W[cols, :].T                                  # [D, C]
            return np.ascontiguousarray(
                wt.reshape(KT, 128, HL, 128).transpose(1, 2, 0, 3)
            ).astype(f16)                                      # [128,HL,KT,128]

        def wslice_v(W):
            wt = W[cols, :].T                                  # [D, C]
            return np.ascontiguousarray(
                wt.reshape(KT, 128, C).transpose(1, 0, 2)
            ).astype(f16)                                      # [128, KT, C]

        wo_t = Wo[:, cols].T                                   # [C, D]
        wo = np.ascontiguousarray(
            wo_t.reshape(HL, 128, D).transpose(1, 0, 2)
        ).astype(f16)                                          # [128, HL, D]

        in_maps.append({
            "xt": xt,
            "wq": wslice(Wq),
            "wk": wslice(Wk),
            "wv": wslice_v(Wv),
            "wo": wo,
            "cos": cos.astype(f16),
            "sin": sin_signed.astype(f16),
            "onem": np.ones((128, 128), dtype=f16),
        })
    return in_maps


def kernel(x, Wq, Wk, Wv, Wo):
    x = np.asarray(x, dtype=np.float32)
    Wq = np.asarray(Wq, dtype=np.float32)
    Wk = np.asarray(Wk, dtype=np.float32)
    Wv = np.asarray(Wv, dtype=np.float32)
    Wo = np.asarray(Wo, dtype=np.float32)

    if "nc" not in _CACHE:
        _CACHE["nc"] = _build_program()
    nc = _CACHE["nc"]

    in_maps = _make_in_maps(x, Wq, Wk, Wv, Wo)
    results = run_bass_kernel_spmd(
        nc, in_maps, core_ids=list(range(N_CORES))
    ).results

    out = np.zeros((B, N, D), dtype=np.float32)
    for c in range(N_CORES):
        out[c // 4] += results[c]["out"].astype(np.float32)
    return out


# revision 49
# speedup vs baseline: 1.0088x; 1.0088x over previous
"""MHSA + RoPE kernel for Trainium2, 8 NeuronCores.

Sharding: data-parallel over batch (B=2) x tensor-parallel over heads
(16 heads -> 4 head-groups of 4). Core c handles batch c//4, heads
[4*(c%4) : 4*(c%4)+4]. Each core computes its partial o_proj output
[N, D]; host sums the 4 partials per batch (the "all-reduce").

Per-core schedule (single TileContext scope, per-head pipeline so the
Tile scheduler can fill attention's ACT-bound PE gaps with the next
head's projection matmuls and keep the PE HAM clock gate warm):

  h=0: k0,q0 proj (+inline RoPE chunks) -> v proj (all heads) -> attn0
  h>0: k_h,q_h proj + RoPE (overlaps attn_{h-1}) -> attn_h
  o_proj at the end (overlaps attn3 via the scheduler).

RoPE is applied to [128,1024] column chunks right after the projection
eviction that produces them, so the rope->scores dependency chain at a
head boundary is ~2us instead of ~10us.

Softmax denominators: an all-ones [128,128] matmul partition-reduces
acc AND broadcasts the result to all partitions in one shot; the
reciprocal runs as reciprocal_approx_fast (single DVE uop chain, ~5x
faster than reciprocal()); the normalize multiply reads a_ps (PSUM) x
bc (SBUF) on DVE.

Everything on-chip is fp16 (same PE rate as bf16, 2x DVE mode, half
the SBUF/DMA of f32, 11-bit mantissa: exp values <= ~200 and softmax
denominators ~3e3 are represented to ~0.05%). PSUM stays f32.

PSUM budget (8 banks): scores [128,1024]x2 bufs = 4, PV accumulator
[128,1024]x1 = 2, shared proj/tail/o_proj pool [128,512]x2 = 2.
o_proj additionally reuses the scores pool slots once attention ends.
"""

import sys

sys.path.insert(0, "/opt/trn_rl_repo")

import numpy as np

import concourse.bass as bass
import concourse.tile as tile
from concourse import bacc, mybir
from concourse.bass_utils import run_bass_kernel_spmd

F32 = mybir.dt.float32
F16 = mybir.dt.float16
MULT = mybir.AluOpType.mult
ADD = mybir.AluOpType.add
EXP = mybir.ActivationFunctionType.Exp
PSUM = bass.MemorySpace.PSUM

B, N, D = 2, 2048, 2048
H, HD = 16, 128
HL = 4            # local heads per core
C = HL * HD       # 512 local head cols
KT = D // 128     # 16 contraction tiles
NB = 4            # n-blocks of 512 for projections
NT = N // 128     # 16 j-tiles
SCALE = float(HD) ** -0.5
N_CORES = 8

_CACHE = {}


def _build_program():
    nc = bacc.Bacc("TRN2", target_bir_lowering=False, debug=False,
                   num_devices=N_CORES)

    xt_d = nc.dram_tensor("xt", [NB, 128, KT, 512], F16, kind="ExternalInput")
    # wq/wk are head-major so head 0's slice (512KB) can load alone in
    # the DMA-bound startup window; heads 1-3 defer until attention 0
    wq_d = nc.dram_tensor("wq", [128, HL, KT, 128], F16,
                          kind="ExternalInput")
    wk_d = nc.dram_tensor("wk", [128, HL, KT, 128], F16,
                          kind="ExternalInput")
    wv_d = nc.dram_tensor("wv", [128, KT, C], F16, kind="ExternalInput")
    wo_d = nc.dram_tensor("wo", [128, HL, D], F16, kind="ExternalInput")
    cos_d = nc.dram_tensor("cos", [128, N], F16, kind="ExternalInput")
    sin_d = nc.dram_tensor("sin", [128, N], F16, kind="ExternalInput")
    onem_d = nc.dram_tensor("onem", [128, 128], F16, kind="ExternalInput")
    out_d = nc.dram_tensor("out", [N, D], F16, kind="ExternalOutput")

    with tile.TileContext(nc) as tc:
        with (
            tc.tile_pool(name="res", bufs=1) as res,
            tc.tile_pool(name="qk", bufs=2) as qkp,
            tc.tile_pool(name="rope", bufs=2) as ropep,
            tc.tile_pool(name="sx", bufs=4) as sxp,
            tc.tile_pool(name="accp", bufs=2) as accp,
            tc.tile_pool(name="pp", bufs=2, space=PSUM) as pp,
            tc.tile_pool(name="sps", bufs=2, space=PSUM) as sps,
            tc.tile_pool(name="aps", bufs=2, space=PSUM) as aps,
        ):
            vv = res.tile([128, NT, C], F16)      # v natural [n, c]
            ao = res.tile([128, HL, N], F16)      # normalized A^T [c, n]
            cos_sb = res.tile([128, N], F16)
            sin_sb = res.tile([128, N], F16)
            onem = res.tile([128, 128], F16)

            def rope_chunk(dst, lo):
                # in-place RoPE on dst[:, lo:lo+1024]; sin sign-folded
                # on host. The d-half swap is a partition shuffle ->
                # SBUF-SBUF DMA.
                sl = slice(lo, lo + 1024)
                tmp = ropep.tile([128, 1024], F16, tag="tmp")
                nc.sync.dma_start(tmp[0:64, :], dst[64:128, sl])
                nc.sync.dma_start(tmp[64:128, :], dst[0:64, sl])
                nc.vector.tensor_tensor(tmp[:], tmp[:], sin_sb[:, sl],
                                        op=MULT)
                nc.vector.tensor_tensor(dst[:, sl], dst[:, sl],
                                        cos_sb[:, sl], op=MULT)
                nc.vector.tensor_tensor(dst[:, sl], dst[:, sl], tmp[:],
                                        op=ADD)

            with tc.tile_pool(name="wp", bufs=1) as wp:
                x_sb = wp.tile([128, NB, KT, 512], F16, tag="x")
                wq_sb = wp.tile([128, HL, KT, 128], F16, tag="wq")
                wk_sb = wp.tile([128, HL, KT, 128], F16, tag="wk")
                wv_sb = wp.tile([128, KT, C], F16, tag="wv")

                # DMA order matches consumption order (k01, q01, v
                # first half, k23, q23, v rest). Half-tensor (1MB)
                # transfers: small per-ktile pieces measured ~200GB/s
                # vs ~430GB/s for large ones, so split no finer than
                # halves, interleaved to spread across DMA queues.
                # Alternate issue between the two HWDGE engines (sync
                # and scalar, both idle at start): each dma_start costs
                # ~600ns of issue time on its engine, so a single-queue
                # burst of 17 serializes ~10us of issue latency.
                startup = [
                    (wk_sb[:, 0], wk_d[:, 0]),
                    (x_sb[:, 0, 0:8], xt_d[0, :, 0:8]),
                    (x_sb[:, 0, 8:16], xt_d[0, :, 8:16]),
                    (wq_sb[:, 0], wq_d[:, 0]),
                    (x_sb[:, 1, 0:8], xt_d[1, :, 0:8]),
                    (x_sb[:, 1, 8:16], xt_d[1, :, 8:16]),
                    (wv_sb[:, 0:8], wv_d[:, 0:8]),
                    (wv_sb[:, 8:16], wv_d[:, 8:16]),
                    (cos_sb[:], cos_d[:]),
                    (sin_sb[:], sin_d[:]),
                    (onem[:], onem_d[:]),
                    (x_sb[:, 2, 0:8], xt_d[2, :, 0:8]),
                    (x_sb[:, 2, 8:16], xt_d[2, :, 8:16]),
                    (x_sb[:, 3, 0:8], xt_d[3, :, 0:8]),
                    (x_sb[:, 3, 8:16], xt_d[3, :, 8:16]),
                    # heads 1-3 of wk/wq aren't touched until the h=1
                    # projections (~65us in): deferred out of the
                    # DMA-bound startup window
                    (wk_sb[:, 1:4], wk_d[:, 1:4]),
                    (wq_sb[:, 1:4], wq_d[:, 1:4]),
                ]
                for dst, src in startup:
                    nc.sync.dma_start(dst, src)

                # Warm the ACT exp table (~2.7us) during the startup
                # DMAs so the first attention exp doesn't eat the load.
                warm = sxp.tile([128, 128], F16, tag="sx")
                nc.scalar.activation(warm[:], cos_sb[:, 0:128], EXP)

                for h in range(HL):
                    # ---- k/q projections for head h: k^T/q^T [d, n],
                    # RoPE chunks inline after the evictions that
                    # complete each 1024-column half. k first so scores
                    # j-tiles unblock as early as possible.
                    qr = qkp.tile([128, N], F16, tag="qr")
                    kr = qkp.tile([128, N], F16, tag="kr")
                    if h == 0:
                        # startup: order matches DMA arrival (wk+x0,
                        # x1, wq, wv, x2, x3) so the PE is never
                        # waiting on a transfer it doesn't need yet,
                        # and attention h0 (which needs kr/qr chunk 0
                        # roped + vv[0..7]) can start early.
                        order = ([("qk", kr, wk_sb, 0),
                                  ("qk", kr, wk_sb, 1),
                                  ("qk", qr, wq_sb, 0),
                                  ("qk", qr, wq_sb, 1)]
                                 + [("v", m) for m in range(8)]
                                 + [("qk", kr, wk_sb, 2),
                                    ("qk", kr, wk_sb, 3),
                                    ("qk", qr, wq_sb, 2),
                                    ("qk", qr, wq_sb, 3)]
                                 + [("v", m) for m in range(8, NT)])
                    else:
                        order = [("qk", kr, wk_sb, nb)
                                 for nb in range(NB)] + \
                                [("qk", qr, wq_sb, nb)
                                 for nb in range(NB)]
                    for item in order:
                        if item[0] == "qk":
                            _, dst, w_sb, nb = item
                            ps = pp.tile([128, 512], F32, tag="pp")
                            for t in range(KT):
                                nc.tensor.matmul(
                                    ps[:],
                                    w_sb[:, h, t, :],
                                    x_sb[:, nb, t, :],
                                    start=(t == 0), stop=(t == KT - 1),
                                )
                            nc.scalar.copy(dst[:, bass.ts(nb, 512)],
                                           ps[:])
                            if nb % 2 == 1:
                                rope_chunk(dst, (nb - 1) * 512)
                        else:
                            # ---- v projection, all heads: v [n, c] ---
                            _, m = item
                            nb, mm = m // 4, m % 4
                            ps = pp.tile([128, 512], F32, tag="pp")
                            for t in range(KT):
                                nc.tensor.matmul(
                                    ps[:],
                                    x_sb[:, nb, t, bass.ts(mm, 128)],
                                    wv_sb[:, t, :],
                                    start=(t == 0), stop=(t == KT - 1),
                                )
                            nc.scalar.copy(vv[:, m, :], ps[:])

                    # ---- attention for head h ------------------------
                    for ih in range(2):
                        ihb = ih * 1024
                        # per-512-col accumulator tiles (1 bank each,
                        # 2 bufs): normalize of f=0 can release its
                        # bank while f=1 still accumulates, halving the
                        # ih-boundary WAR stall.
                        a_ps0 = aps.tile([128, 512], F32, tag="a")
                        a_ps1 = aps.tile([128, 512], F32, tag="a")
                        a_ps = (a_ps0, a_ps1)
                        acc = accp.tile([128, 1024], F16, tag="acc")
                        # For the very last tail (nothing left to fill
                        # PE gaps with), shorten the post-exp critical
                        # chain: the last two j-tiles skip the serial
                        # DVE accumulate and instead contribute to the
                        # denominator via PSUM-accumulated matmuls.
                        last_tail = (h == HL - 1 and ih == 1)
                        tail_exps = []
                        for j in range(NT):
                            s_ps = sps.tile([128, 1024], F32, tag="s")
                            for f in range(2):
                                nc.tensor.matmul(
                                    s_ps[:, bass.ts(f, 512)],
                                    kr[:, bass.ts(j, 128)],
                                    qr[:, ihb + f * 512:
                                        ihb + (f + 1) * 512],
                                    start=True, stop=True,
                                )
                            s_exp = sxp.tile([128, 1024], F16, tag="sx")
                            nc.scalar.activation(s_exp[:], s_ps[:], EXP,
                                                 scale=SCALE)
                            if last_tail and j >= NT - 2:
                                tail_exps.append(s_exp)
                            elif j == 0:
                                nc.vector.tensor_copy(acc[:], s_exp[:])
                            else:
                                nc.vector.tensor_tensor(acc[:], acc[:],
                                                        s_exp[:], op=ADD)
                            for f in range(2):
                                nc.tensor.matmul(
                                    a_ps[f][:],
                                    vv[:, j, bass.ts(h, 128)],
                                    s_exp[:, bass.ts(f, 512)],
                                    start=(j == 0), stop=(j == NT - 1),
                                )
                        # softmax denominators: the all-ones [128,128]
                        # matmul partition-reduces acc AND broadcasts
                        # den[i] to every partition; fast approx
                        # reciprocal; normalize on DVE.
                        for f in range(2):
                            # use a scores-pool slot (fast-cycling, not
                            # the pp slots that next-head proj groups
                            # need to fill the ih-boundary gap)
                            den_ps = sps.tile([128, 512], F32, tag="s")
                            nc.tensor.matmul(den_ps[:], onem[:],
                                             acc[:, bass.ts(f, 512)],
                                             start=True,
                                             stop=not tail_exps)
                            for ti, te in enumerate(tail_exps):
                                nc.tensor.matmul(
                                    den_ps[:], onem[:],
                                    te[:, bass.ts(f, 512)],
                                    start=False,
                                    stop=(ti == len(tail_exps) - 1))
                            bc_sb = accp.tile([128, 512], F32, tag="bc")
                            with nc.allow_low_precision(
                                    reason="softmax denominators: approx "
                                           "recip is ~51 ULP"):
                                nc.vector.reciprocal_approx_fast(
                                    out=bc_sb[:], in_=den_ps[:])
                            nc.vector.tensor_tensor(
                                ao[:, h, ihb + f * 512:
                                   ihb + (f + 1) * 512],
                                a_ps[f][:], bc_sb[:],
                                op=MULT)

            # ---- o_proj (wo/st pools reuse the closed wp space) ------
            with (
                tc.tile_pool(name="op", bufs=1) as op,
                tc.tile_pool(name="stp", bufs=3) as stp,
            ):
                wo_sb = op.tile([128, HL, D], F16, tag="wo")
                nc.sync.dma_start(wo_sb[:], wo_d[:])
                for m in range(NT):
                    st = stp.tile([128, D], F16, tag="st")
                    for f in range(4):
                        # spread o_ps over the pp pool and the (now
                        # idle) scores pool for 4-deep PSUM pipelining
                        if f % 2 == 0:
                            o_ps = pp.tile([128, 512], F32, tag="pp")
                        else:
                            o_ps = sps.tile([128, 512], F32, tag="s")
                        for ct in range(HL):
                            nc.tensor.matmul(
                                o_ps[:],
                                ao[:, ct, bass.ts(m, 128)],
                                wo_sb[:, ct, bass.ts(f, 512)],
                                start=(ct == 0), stop=(ct == HL - 1),
                            )
                        # alternate eviction engines so neither paces
                        # o_proj
                        if f % 2 == 0:
                            nc.scalar.copy(st[:, bass.ts(f, 512)],
                                           o_ps[:])
                        else:
                            nc.vector.tensor_copy(st[:, bass.ts(f, 512)],
                                                  o_ps[:])
                        # store each 512-col slice as soon as its
                        # eviction lands (shrinks the final-store tail)
                        nc.sync.dma_start(
                            out_d[bass.ts(m, 128), bass.ts(f, 512)],
                            st[:, bass.ts(f, 512)])

    nc.compile()
    return nc


def _rope_tables():
    inv_freq = 1.0 / (10000.0 ** (np.arange(0, HD, 2, dtype=np.float32) / HD))
    pos = np.arange(N, dtype=np.float32)
    freqs = pos[:, None] * inv_freq[None, :]          # [N, HD/2]
    emb = np.concatenate([freqs, freqs], axis=-1)     # [N, HD]
    cos = np.cos(emb).astype(np.float32).T.copy()     # [HD, N]
    sin = np.sin(emb).astype(np.float32).T.copy()     # [HD, N]
    sin_signed = sin.copy()
    sin_signed[0:64] *= -1.0
    return cos, sin_signed


def _make_in_maps(x, Wq, Wk, Wv, Wo):
    cos, sin_signed = _rope_tables()
    f16 = np.float16

    in_maps = []
    for c in range(N_CORES):
        b, hg = c // 4, c % 4
        cols = slice(C * hg, C * hg + C)
        xT = np.ascontiguousarray(x[b].T)                      # [D, N]
        xt = np.ascontiguousarray(
            xT.reshape(KT, 128, NB, 512).transpose(2, 1, 0, 3)
        ).astype(f16)                                          # [NB,128,KT,512]

        def wslice(W):
            wt = W[cols, :].T                                  # [D, C]
            return np.ascontiguousarray(
                wt.reshape(KT, 128, HL, 128).transpose(1, 2, 0, 3)
            ).astype(f16)                                      # [128,HL,KT,128]

        def wslice_v(W):
            wt = W[cols, :].T                                  # [D, C]
            return np.ascontiguousarray(
                wt.reshape(KT, 128, C).transpose(1, 0, 2)
            ).astype(f16)                                      # [128, KT, C]

        wo_t = Wo[:, cols].T                                   # [C, D]
        wo = np.ascontiguousarray(
            wo_t.reshape(HL, 128, D).transpose(1, 0, 2)
        ).astype(f16)                                          # [128, HL, D]

        in_maps.append({
            "xt": xt,
            "wq": wslice(Wq),
            "wk": wslice(Wk),
            "wv": wslice_v(Wv),
            "wo": wo,
            "cos": cos.astype(f16),
            "sin": sin_signed.astype(f16),
            "onem": np.ones((128, 128), dtype=f16),
        })
    return in_maps


def kernel(x, Wq, Wk, Wv, Wo):
    x = np.asarray(x, dtype=np.float32)
    Wq = np.asarray(Wq, dtype=np.float32)
    Wk = np.asarray(Wk, dtype=np.float32)
    Wv = np.asarray(Wv, dtype=np.float32)
    Wo = np.asarray(Wo, dtype=np.float32)

    if "nc" not in _CACHE:
        _CACHE["nc"] = _build_program()
    nc = _CACHE["nc"]

    in_maps = _make_in_maps(x, Wq, Wk, Wv, Wo)
    results = run_bass_kernel_spmd(
        nc, in_maps, core_ids=list(range(N_CORES))
    ).results

    out = np.zeros((B, N, D), dtype=np.float32)
    for c in range(N_CORES):
        out[c // 4] += results[c]["out"].astype(np.float32)
    return out
